# revision 2
# baseline (speedup 1.0000x reference)
"""Gemma-style transformer block (GQA + sliding-window attention + gated-GELU
MLP) on 8 Trainium2 NeuronCores — v2.

Sharding (Megatron + sequence-parallel), same as v1:
  - Attention TP over heads (core c: q heads {2c,2c+1}, kv head c);
    per-batch ReduceScatter (bf16) leaves core c tokens [128c,128c+128).
  - MLP TP over hidden dim; AllGather h2^T (bf16), ReduceScatter after
    down-proj (bf16).

v2 changes vs v1:
  - All matmuls in bf16 (PSUM accum fp32); all collective payloads bf16.
  - Pre-attn RMSNorm eliminated algebraically: premul folded into wqkv
    (host), q/k row scale cancels in qk-norm, v row scale r_t applied at
    v-copy (host-computed r_t), qk-norm row scales folded into the tanh
    softcap scale (q, per-partition) and a k row scale before transpose.
  - qk-norm col scales (incl. H^-0.5) folded into host rope tables.
  - All transposes moved off the PE onto the DMA xbar (16-bit transpose).
  - Elementwise work spread across scalar/pool(gpsimd)/DVE engines.
  - attn_out residual kept in SBUF across the MLP phase.
"""
import sys

sys.path.insert(0, "/opt/trn_rl_repo")

import numpy as np
import ml_dtypes

import concourse.bass as bass
import concourse.mybir as mybir
import concourse.tile as tile
from concourse import bacc

F32 = mybir.dt.float32
BF16 = mybir.dt.bfloat16
AF = mybir.ActivationFunctionType
OP = mybir.AluOpType
BF = ml_dtypes.bfloat16

B, T, D = 2, 1024, 2048
NQ, KV, H, HID = 16, 8, 128, 8192
WINDOW, CAP = 512, 50.0
KMASK = -2.3819763e38
EPS = 1e-6
ROPE_BASE = 10000.0
NCORES = 8
DT = D // 128          # 16 contraction tiles over D
TB = T // 128          # 8 token blocks per batch
RG = [list(range(NCORES))]

# mask-row offsets per pair p (keys = nu*128); layout [128, 4608]
_PAIR = []
_off = 0
for _p in range(4):
    _ukb0 = max(0, 2 * _p - 4)
    _nu = 2 * _p + 2 - _ukb0
    _PAIR.append((_ukb0, _nu, _off))
    _off += 2 * _nu * 128
MASKW = _off           # 4608


def _midx(qb, kb):
    if kb > qb:
        return 0       # future block: fully masked
    if kb == qb:
        return 1       # causal lower-tri (incl diag)
    if kb == qb - 4:
        return 3       # window tail: strict upper-tri allowed
    if kb < qb - 4:
        return 0       # fully outside window
    return 2           # fully inside window: no mask


def _build_program(reps=1, single=False, do_attn=True, do_mlp=True,
                   fake_coll=False, fake_rs1=False, fake_ag=False,
                   fake_rs2=False):
    nc = bacc.Bacc("TRN2", target_bir_lowering=False, debug=False,
                   enable_asserts=True,
                   num_devices=(1 if single else NCORES))

    def din(name, shape, dt=F32):
        return nc.dram_tensor(name, shape, dt, kind="ExternalInput").ap()

    xbf = din("xbf", [B * T, D], BF16)
    xsh = din("xsh", [2 * 128, D])
    wqkv = din("wqkv", [D, 512], BF16)          # [D, q0|q1|k|v]
    ow = din("ow", [256, D], BF16)              # [2*H rows, D]
    gw = din("gw", [D, 1024], BF16)
    uw = din("uw", [D, 1024], BF16)
    dw = din("dw", [1024, D], BF16)
    tabA = din("tabA", [B * T, 192], BF16)      # cos * mul_first
    tabB = din("tabB", [B * T, 192], BF16)      # sin * mul_second
    tabC = din("tabC", [B * T, 192], BF16)      # cos * mul_second
    tabD = din("tabD", [B * T, 192], BF16)      # sin * mul_first
    rt = din("rt", [B, 128, TB])                # pre-attn rms scalar
    maskq = din("maskq", [128, 4, 128], BF16)   # canonical^T / CAP
    postattnmul = din("postattnmul", [D], BF16)
    preffwmul = din("preffwmul", [D], BF16)
    postffwmul = din("postffwmul", [D], BF16)

    out = nc.dram_tensor("out", [2 * 128, D], F32, kind="ExternalOutput").ap()

    if single:
        fake_coll = True
    if fake_coll:
        fake_rs1 = fake_ag = fake_rs2 = True

    with tile.TileContext(nc) as tc:
        for _ in range(reps):
            _body(nc, tc, xbf=xbf, xsh=xsh, wqkv=wqkv, ow=ow, gw=gw, uw=uw,
                  dw=dw, tabA=tabA, tabB=tabB, tabC=tabC, tabD=tabD, rt=rt,
                  maskq=maskq, postattnmul=postattnmul,
                  preffwmul=preffwmul, postffwmul=postffwmul, out=out,
                  do_attn=do_attn, do_mlp=do_mlp, fake_rs1=fake_rs1,
                  fake_ag=fake_ag, fake_rs2=fake_rs2)
    nc.compile()
    return nc


def _body(nc, tc, *, xbf, xsh, wqkv, ow, gw, uw, dw, tabA, tabB, tabC, tabD,
          rt, maskq, postattnmul, preffwmul, postffwmul, out,
          do_attn, do_mlp, fake_rs1, fake_ag, fake_rs2):
    from contextlib import ExitStack

    def _rscatter(in_ap, out_ap, fake):
        if fake:
            nrows = out_ap.shape[0]
            nc.sync.dma_start(out_ap, in_ap[0:nrows, :])
        else:
            nc.gpsimd.collective_compute(
                "ReduceScatter", OP.add, replica_groups=RG,
                ins=[in_ap.opt()], outs=[out_ap.opt()])

    def _agather(in_ap, out_ap):
        if fake_ag:
            nrows = in_ap.shape[0]
            for r in range(NCORES):
                nc.sync.dma_start(out_ap[r * nrows:(r + 1) * nrows, :], in_ap)
        else:
            nc.gpsimd.collective_compute(
                "AllGather", OP.bypass, replica_groups=RG,
                ins=[in_ap.opt()], outs=[out_ap.opt()])

    est = ExitStack()
    with est:
        # ----- long-lived pools -----
        consts = est.enter_context(tc.tile_pool(name="consts", bufs=1))
        bcast = est.enter_context(tc.tile_pool(name="bcast", bufs=3))
        small = est.enter_context(tc.tile_pool(name="small", bufs=32))
        sqd = est.enter_context(tc.tile_pool(name="sqd", bufs=1))
        bwork = est.enter_context(tc.tile_pool(name="bwork", bufs=3))
        resid = est.enter_context(tc.tile_pool(name="resid", bufs=1))
        xTp = est.enter_context(tc.tile_pool(name="xTp", bufs=1))
        dram = est.enter_context(tc.tile_pool(name="dram", bufs=1,
                                              space="DRAM"))

        eps_t = consts.tile([128, 1], F32)
        nc.vector.memset(eps_t[:], EPS)
        epsc2_t = consts.tile([128, 1], F32)
        nc.vector.memset(epsc2_t[:], EPS * CAP * CAP)

        postattn_bc = bcast.tile([128, D], BF16, tag="bc")
        nc.sync.dma_start(postattn_bc[:], bass.AP(
            tensor=postattnmul.tensor, offset=postattnmul.offset,
            ap=[[0, 128], *postattnmul.ap]))
        preffw_bc = bcast.tile([128, D], BF16, tag="bc")
        nc.sync.dma_start(preffw_bc[:], bass.AP(
            tensor=preffwmul.tensor, offset=preffwmul.offset,
            ap=[[0, 128], *preffwmul.ap]))
        postffw_bc = bcast.tile([128, D], BF16, tag="bc")
        nc.sync.dma_start(postffw_bc[:], bass.AP(
            tensor=postffwmul.tensor, offset=postffwmul.offset,
            ap=[[0, 128], *postffwmul.ap]))

        # residual (attn_out) kept on-chip across the MLP phase
        aores = resid.tile([128, B, D], BF16)

        # DRAM intermediates (collective payloads all bf16)
        o_part = dram.tile([B * T, D], BF16)
        rs1 = dram.tile([B * 128, D], BF16)
        ag_in = dram.tile([B * 128, D], BF16)
        ag_sp = "Local" if fake_ag else "Shared"
        ag_outs = [
            dram.tile([T, D], BF16, addr_space=ag_sp,
                      tag=f"ag_out{b}", name=f"ag_out{b}")
            for b in range(B)
        ]
        RS2C = 2  # RS2 chunks per batch (D/RS2C wide)
        rs2w = D // RS2C
        mlp_chs = [dram.tile([B * T, rs2w], BF16, tag=f"mlpc{c}",
                             name=f"mlpc{c}") for c in range(RS2C)]
        rs2_chs = [dram.tile([B * 128, rs2w], BF16, tag=f"rs2c{c}",
                             name=f"rs2c{c}") for c in range(RS2C)]

        def _bchain(b):
            """post-attn norm + residual (SBUF) + pre-ffw norm + h2T + AG."""
            rst = bwork.tile([128, D], BF16, tag="bw", name="rst")
            nc.sync.dma_start(rst[:], rs1[b * 128:(b + 1) * 128, :])
            ss = small.tile([128, 1], F32)
            sq = sqd.tile([128, D], BF16, tag="sq")
            nc.scalar.activation(sq[:], rst[:], AF.Square, accum_out=ss[:])
            rs_t = small.tile([128, 1], F32)
            nc.scalar.activation(rs_t[:], ss[:], AF.Sqrt, scale=1.0 / D,
                                 bias=eps_t[:])
            nc.vector.reciprocal(rs_t[:], rs_t[:])
            ao = bwork.tile([128, D], F32, tag="bw", name="ao")
            nc.vector.scalar_tensor_tensor(
                out=ao[:], in0=rst[:], scalar=rs_t[:], in1=postattn_bc[:],
                op0=OP.mult, op1=OP.mult)
            xst = bwork.tile([128, D], F32, tag="bw", name="xst")
            nc.sync.dma_start(xst[:], xsh[b * 128:(b + 1) * 128, :])
            nc.vector.tensor_add(aores[:, b, :], ao[:], xst[:])
            # pre-ffw rms -> h2 (bf16)
            ss2 = small.tile([128, 1], F32)
            sq2 = sqd.tile([128, D], BF16, tag="sq")
            nc.scalar.activation(sq2[:], aores[:, b, :], AF.Square,
                                 accum_out=ss2[:])
            r2_t = small.tile([128, 1], F32)
            nc.scalar.activation(r2_t[:], ss2[:], AF.Sqrt, scale=1.0 / D,
                                 bias=eps_t[:])
            nc.vector.reciprocal(r2_t[:], r2_t[:])
            h2 = bwork.tile([128, D], BF16, tag="bw", name="h2")
            nc.vector.scalar_tensor_tensor(
                out=h2[:], in0=aores[:, b, :], scalar=r2_t[:],
                in1=preffw_bc[:], op0=OP.mult, op1=OP.mult)
            # AllGather the untransposed h2 token-shard; receivers
            # transpose from DRAM via the xbar.
            nc.sync.dma_start(ag_in[b * 128:(b + 1) * 128, :], h2[:])
            _agather(ag_in[b * 128:(b + 1) * 128, :], ag_outs[b][:, :])

        # =================== ATTENTION (TP over heads) ===================
        with ExitStack() as ascope:
          if do_attn:
            ap_ = {}
            for nm, args in [
                ("wqkvp", dict(bufs=1)), ("owp", dict(bufs=1)),
                ("maskp", dict(bufs=1)), ("tabp", dict(bufs=1)),
                ("rtp", dict(bufs=2)),
                ("pqs", dict(bufs=2)), ("ro", dict(bufs=2)),
                ("qkrow", dict(bufs=3)), ("qT", dict(bufs=2)),
                ("kT", dict(bufs=2)), ("vp", dict(bufs=2)),
                ("t1p", dict(bufs=4)), ("pbf", dict(bufs=8)),
                ("encn", dict(bufs=3)), ("encp", dict(bufs=2)),
                ("obp", dict(bufs=3)),
            ]:
                ap_[nm] = ascope.enter_context(tc.tile_pool(name=nm, **args))
            psA = ascope.enter_context(
                tc.tile_pool(name="psA", bufs=2, space="PSUM"))
            psP = ascope.enter_context(
                tc.tile_pool(name="psP", bufs=3, space="PSUM"))
            psE = ascope.enter_context(
                tc.tile_pool(name="psE", bufs=2, space="PSUM"))

            wqkv_sb = ap_["wqkvp"].tile([128, DT, 512], BF16)
            for dt in range(DT):
                nc.sync.dma_start(wqkv_sb[:, dt, :],
                                  wqkv[dt * 128:(dt + 1) * 128, :])
            ow_sb = ap_["owp"].tile([128, 2, 4, 512], BF16)
            for hh in range(2):
                for ch in range(4):
                    nc.scalar.dma_start(
                        ow_sb[:, hh, ch, :],
                        ow[hh * 128:(hh + 1) * 128, ch * 512:(ch + 1) * 512])
            maskq_sb = ap_["maskp"].tile([128, 4, 128], BF16)
            nc.scalar.dma_start(maskq_sb[:], maskq[:])

            def _a_prologue(b, on_scalar_q=False):
                st = {}
                tabs = []
                for ti_, tab in enumerate((tabA, tabB, tabC, tabD)):
                    tt = ap_["tabp"].tile([128, TB, 3, 64], BF16,
                                          tag=f"tab{ti_}")
                    nc.sync.dma_start(
                        tt[:], tab[b * T:(b + 1) * T, :].rearrange(
                            "(tb p) x -> p tb x", p=128))
                    tabs.append(tt)
                st["tabs"] = tabs
                rt_sb = ap_["rtp"].tile([128, TB], F32, tag="rt")
                nc.sync.dma_start(rt_sb[:], rt[b, :, :])
                st["rt"] = rt_sb
                st["qT"] = ap_["qT"].tile([128, 2, T], BF16, name="qTt")
                st["kT"] = ap_["kT"].tile([128, T], BF16, name="kTt")
                v_sb = ap_["vp"].tile([128, TB, 129], BF16)
                nc.vector.memset(v_sb[:, :, 128:129], 1.0)
                st["v"] = v_sb
                xT = xTp.tile([128, DT, T], BF16)
                dq = nc.scalar if on_scalar_q else nc.sync
                for dt in range(DT):
                    dq.dma_start_transpose(
                        xT[:, dt, :],
                        xbf[b * T:(b + 1) * T, dt * 128:(dt + 1) * 128])
                st["xT"] = xT
                return st

            def _a_tb(b, st, tb):
                xT, tabs, rt_sb = st["xT"], st["tabs"], st["rt"]
                qT, kT, v_sb = st["qT"], st["kT"], st["v"]
                pq = psA.tile([128, 4, 128], F32, tag="mm")
                for dt in range(DT):
                    nc.tensor.matmul(
                        pq[:], xT[:, dt, tb * 128:(tb + 1) * 128],
                        wqkv_sb[:, dt, :],
                        start=(dt == 0), stop=(dt == DT - 1))
                # stage q0|q1|k to SBUF for pool-engine rope (DVE copy)
                pqs = ap_["pqs"].tile([128, 3, 128], F32)
                nc.vector.tensor_copy(pqs[:], pq[:, 0:3, :])
                # row-scales r_q0, r_q1, r_k: squares on DVE as pq*pqs
                rqk_t = []
                for hd in range(3):
                    ss = small.tile([128, 1], F32)
                    sq = sqd.tile([128, 128], BF16, tag="sqa")
                    nc.vector.scalar_tensor_tensor(
                        out=sq[:], in0=pq[:, hd, :], scalar=1.0,
                        in1=pqs[:, hd, :], op0=OP.mult, op1=OP.mult,
                        accum_out=ss[:])
                    dst = small.tile([128, 1], F32, name=f"rqk{hd}")
                    nc.scalar.activation(dst[:], ss[:], AF.Sqrt,
                                         scale=1.0 / H, bias=eps_t[:])
                    nc.vector.reciprocal(dst[:], dst[:])
                    rqk_t.append(dst)
                ro3 = ap_["ro"].tile([128, 3, 128], F32)
                tm1 = ap_["ro"].tile([128, 3, 64], F32, name="tm1")
                tm2 = ap_["ro"].tile([128, 3, 64], F32, name="tm2")
                nc.gpsimd.tensor_mul(tm1[:], pqs[:, :, 0:64],
                                     tabs[0][:, tb, :, :])
                nc.gpsimd.tensor_mul(tm2[:], pqs[:, :, 64:128],
                                     tabs[1][:, tb, :, :])
                nc.gpsimd.tensor_sub(ro3[:, :, 0:64], tm1[:], tm2[:])
                tm3 = ap_["ro"].tile([128, 3, 64], F32, name="tm3")
                tm4 = ap_["ro"].tile([128, 3, 64], F32, name="tm4")
                nc.gpsimd.tensor_mul(tm3[:], pqs[:, :, 64:128],
                                     tabs[2][:, tb, :, :])
                nc.gpsimd.tensor_mul(tm4[:], pqs[:, :, 0:64],
                                     tabs[3][:, tb, :, :])
                nc.gpsimd.tensor_add(ro3[:, :, 64:128], tm3[:], tm4[:])
                # q/k: row-scale (r_q, r_k) folded into the bf16 cast
                for hd in range(3):
                    qrow = ap_["qkrow"].tile([128, 128], BF16, tag="qk")
                    nc.vector.tensor_scalar_mul(qrow[:], ro3[:, hd, :],
                                                rqk_t[hd][:])
                    dst = (qT[:, hd, tb * 128:(tb + 1) * 128] if hd < 2
                           else kT[:, tb * 128:(tb + 1) * 128])
                    nc.sync.dma_start_transpose(dst, qrow[:])
                # v with r_t row scale (psum->sbuf bf16)
                nc.vector.tensor_scalar_mul(v_sb[:, tb, 0:128],
                                            pq[:, 3, :],
                                            rt_sb[:, tb:tb + 1])

            # ---- phase B: transposed-logits banded attention ----
            def _logits(st, p, h):
                qT, kT = st["qT"], st["kT"]
                ukb0, nu, moff = _PAIR[p]
                pbfs = []
                for m in range(nu // 2):
                    psl = psP.tile([128, 2, 256], F32)
                    for jj in range(2):
                        kb = ukb0 + 2 * m + jj
                        nc.tensor.matmul(
                            psl[:, jj, :],
                            kT[:, kb * 128:(kb + 1) * 128],
                            qT[:, h, p * 256:(p + 1) * 256],
                            start=True, stop=True)
                    t1 = ap_["t1p"].tile([128, 2, 256], F32)
                    nc.scalar.activation(t1[:], psl[:], AF.Tanh,
                                         scale=1.0 / CAP)
                    # in-place canonical-tile mask adds; maskq is /CAP
                    for jj in range(2):
                        for qi in range(2):
                            v = _midx(2 * p + qi, ukb0 + 2 * m + jj)
                            if v != 2:
                                sl = t1[:, jj, qi * 128:(qi + 1) * 128]
                                nc.gpsimd.tensor_add(
                                    sl, sl, maskq_sb[:, v, :])
                    pbf = ap_["pbf"].tile([128, 2, 256], BF16)
                    nc.scalar.activation(pbf[:], t1[:], AF.Exp,
                                         scale=float(CAP))
                    pbfs.append(pbf)
                return pbfs

            def _pv_enc(st, p, h, pbfs, enc_sb):
                v_sb = st["v"]
                ukb0, nu, moff = _PAIR[p]
                for qi in range(2):
                    js = [j for j in range(nu)
                          if _midx(2 * p + qi, ukb0 + j) != 0]
                    pe = psE.tile([128, 129], F32)
                    for i, j in enumerate(js):
                        nc.tensor.matmul(
                            pe[:],
                            pbfs[j // 2][:, j % 2,
                                         qi * 128:(qi + 1) * 128],
                            v_sb[:, ukb0 + j, :],
                            start=(i == 0), stop=(i == len(js) - 1))
                    denr = small.tile([128, 1], F32)
                    nc.vector.reciprocal(denr[:], pe[:, 128:129])
                    encn = ap_["encn"].tile([128, 128], BF16)
                    nc.vector.tensor_scalar_mul(encn[:], pe[:, 0:128],
                                                denr[:])
                    nc.sync.dma_start_transpose(
                        enc_sb[:, h, qi * 128:(qi + 1) * 128], encn[:])

            def _oproj(b, p, enc_sb):
                for qi in range(2):
                    qb = 2 * p + qi
                    ob = ap_["obp"].tile([128, D], BF16, tag="ob")
                    for ch in range(4):
                        po = psA.tile([128, 4, 128], F32, tag="mm")
                        for h in range(2):
                            nc.tensor.matmul(
                                po[:],
                                enc_sb[:, h, qi * 128:(qi + 1) * 128],
                                ow_sb[:, h, ch, :],
                                start=(h == 0), stop=(h == 1))
                        nc.vector.tensor_copy(
                            ob[:, ch * 512:(ch + 1) * 512], po[:])
                    nc.sync.dma_start(
                        o_part[b * T + qb * 128:b * T + (qb + 1) * 128,
                               :], ob[:])

            def _b_units(b, st, cb=None):
                units = [(p, h) for p in range(4) for h in range(2)]
                pending = None
                enc_cur = None
                for ui, (p, h) in enumerate(units):
                    if h == 0:
                        enc_cur = ap_["encp"].tile([128, 2, 256], BF16)
                    pbfs = _logits(st, p, h)
                    if pending is not None:
                        pp, ph, ppbfs, penc = pending
                        _pv_enc(st, pp, ph, ppbfs, penc)
                        if ph == 1:
                            _oproj(b, pp, penc)
                    pending = (p, h, pbfs, enc_cur)
                    if cb is not None:
                        cb(ui)
                pp, ph, ppbfs, penc = pending
                _pv_enc(st, pp, ph, ppbfs, penc)
                _oproj(b, pp, penc)

            # batch 0 phase A, then b0 phase B interleaved with b1 phase A
            st0 = _a_prologue(0)
            for tb in range(TB):
                _a_tb(0, st0, tb)
            st1 = _a_prologue(1, on_scalar_q=True)
            _b_units(0, st0, cb=lambda ui: _a_tb(1, st1, ui))
            _rscatter(o_part[0:T, :], rs1[0:128, :], fake_rs1)
            _bchain(0)
            # prefetch batch-0's gathered-h2 transpose into the xT ring slot
            # (free once b1's qkv is done); runs during b1 phase B
            h2Tf0 = xTp.tile([128, DT, T], BF16, name="h2Tf0",
                                    tag="xT")
            for dt in range(DT):
                nc.scalar.dma_start_transpose(
                    h2Tf0[:, dt, :], ag_outs[0][:, dt * 128:(dt + 1) * 128])
            _b_units(1, st1)
            _rscatter(o_part[T:2 * T, :], rs1[128:2 * 128, :], fake_rs1)
            _bchain(1)

        # =================== MLP (TP over hidden dim) ===================
        with ExitStack() as mscope:
          if do_mlp:
            mp_ = {}
            for nm, args in [
                ("h2Tp", dict(bufs=1)), ("wst", dict(bufs=2)),
                ("actp", dict(bufs=1)), ("dwp", dict(bufs=2)),
                ("gelp", dict(bufs=2)), ("mbp", dict(bufs=2)),
            ]:
                mp_[nm] = mscope.enter_context(tc.tile_pool(name=nm, **args))
            psM = mscope.enter_context(
                tc.tile_pool(name="psM", bufs=4, space="PSUM"))

            # b0's h2^T was prefetched into the xT ring during attention
            h2Tf = [h2Tf0,
                    mp_["h2Tp"].tile([128, DT, 1024], BF16, name="h2Tf1")]
            def _load_guw(hc, tag, nb=None):
                gw_t = mp_["wst"].tile([128, DT, 128], BF16, tag=f"{tag}g",
                                       bufs=nb)
                gv = gw[:, hc * 128:(hc + 1) * 128].rearrange(
                    "(dt p) h -> p dt h", p=128)
                uw_t = mp_["wst"].tile([128, DT, 128], BF16, tag=f"{tag}u",
                                       bufs=nb)
                uv = uw[:, hc * 128:(hc + 1) * 128].rearrange(
                    "(dt p) h -> p dt h", p=128)
                nc.sync.dma_start(gw_t[:], gv[:])
                nc.sync.dma_start(uw_t[:], uv[:])
                return gw_t, uw_t

            # preload first two hid chunks' weights (overlaps attn tail)
            pre_w = {hc: _load_guw(hc, f"pw{hc}", nb=1) for hc in range(2)}
            for dt in range(DT):
                nc.sync.dma_start_transpose(
                    h2Tf[1][:, dt, :],
                    ag_outs[1][:, dt * 128:(dt + 1) * 128])
            actT = [mp_["actp"].tile([128, 8, 1024], BF16,
                                     tag=f"actT{b}", name=f"actT{b}")
                    for b in range(B)]
            for hc in range(8):
                gw_t, uw_t = pre_w[hc] if hc in pre_w else _load_guw(hc, "w")
                for b in range(B):
                    for tch in range(2):
                        psg = psM.tile([128, 512], F32, tag="psm")
                        psu = psM.tile([128, 512], F32, tag="psm")
                        for dt in range(DT):
                            nc.tensor.matmul(
                                psg[:], gw_t[:, dt, :],
                                h2Tf[b][:, dt, tch * 512:(tch + 1) * 512],
                                start=(dt == 0), stop=(dt == DT - 1))
                        for dt in range(DT):
                            nc.tensor.matmul(
                                psu[:], uw_t[:, dt, :],
                                h2Tf[b][:, dt, tch * 512:(tch + 1) * 512],
                                start=(dt == 0), stop=(dt == DT - 1))
                        gel = mp_["gelp"].tile([128, 512], BF16, tag="gel")
                        nc.scalar.activation(gel[:], psg[:],
                                             AF.Gelu_apprx_tanh)
                        nc.vector.tensor_mul(
                            actT[b][:, hc, tch * 512:(tch + 1) * 512],
                            gel[:], psu[:])
            # down-projection, streamed per 512-wide D sub-chunk
            for ch in range(RS2C):
                for sc in range(rs2w // 512):
                    dw_t = mp_["dwp"].tile([128, 8, 512], BF16)
                    for hc in range(8):
                        nc.sync.dma_start(
                            dw_t[:, hc, :],
                            dw[hc * 128:(hc + 1) * 128,
                               ch * rs2w + sc * 512:
                               ch * rs2w + (sc + 1) * 512])
                    for b in range(B):
                        for tb2 in range(TB // 2):
                            mb = mp_["mbp"].tile([128, 2, 512], BF16,
                                                 tag="mb")
                            for half in range(2):
                                tb = tb2 * 2 + half
                                psd = psM.tile([128, 512], F32, tag="psm")
                                for hc in range(8):
                                    nc.tensor.matmul(
                                        psd[:],
                                        actT[b][:, hc,
                                                tb * 128:(tb + 1) * 128],
                                        dw_t[:, hc, :],
                                        start=(hc == 0), stop=(hc == 7))
                                if half == 0:
                                    nc.scalar.activation(mb[:, half, :],
                                                         psd[:], AF.Copy)
                                else:
                                    nc.vector.tensor_copy(mb[:, half, :],
                                                          psd[:])
                            nc.sync.dma_start(
                                mlp_chs[ch][b * T + tb2 * 256:
                                            b * T + (tb2 + 1) * 256,
                                            sc * 512:(sc + 1) * 512]
                                .rearrange("(u p) w -> p u w", p=128),
                                mb[:])
                for b in range(B):
                    _rscatter(mlp_chs[ch][b * T:(b + 1) * T, :],
                              rs2_chs[ch][b * 128:(b + 1) * 128, :],
                              fake_rs2)
        if not do_mlp:
            for ch in range(RS2C):
                for b in range(B):
                    nc.sync.dma_start(
                        rs2_chs[ch][b * 128:(b + 1) * 128, :],
                        rs1[b * 128:(b + 1) * 128,
                            ch * rs2w:(ch + 1) * rs2w])

        # =================== final norm + residual ===================
        for b in range(B):
            rst = bwork.tile([128, D], BF16, tag="bw")
            for ch in range(RS2C):
                nc.sync.dma_start(rst[:, ch * rs2w:(ch + 1) * rs2w],
                                  rs2_chs[ch][b * 128:(b + 1) * 128, :])
            ss = small.tile([128, 1], F32)
            sq = sqd.tile([128, D], BF16, tag="sq")
            nc.scalar.activation(sq[:], rst[:], AF.Square, accum_out=ss[:])
            rs_t = small.tile([128, 1], F32)
            nc.scalar.activation(rs_t[:], ss[:], AF.Sqrt, scale=1.0 / D,
                                 bias=eps_t[:])
            nc.vector.reciprocal(rs_t[:], rs_t[:])
            tmp = bwork.tile([128, D], F32, tag="bw")
            nc.vector.scalar_tensor_tensor(
                out=tmp[:], in0=rst[:], scalar=rs_t[:], in1=postffw_bc[:],
                op0=OP.mult, op1=OP.mult)
            nc.vector.tensor_add(tmp[:], tmp[:], aores[:, b, :])
            for g in range(4):
                nc.sync.dma_start(
                    out[b * 128:(b + 1) * 128, g * 512:(g + 1) * 512],
                    tmp[:, g * 512:(g + 1) * 512])


# ---------------------------------------------------------------------------
# host side
# ---------------------------------------------------------------------------

_NC = None


def _get_nc():
    global _NC
    if _NC is None:
        _NC = _build_program()
    return _NC


def _host_prep(inputs):
    x = np.ascontiguousarray(np.asarray(inputs["x"], dtype=np.float32))
    seg = np.asarray(inputs["segment_pos"], dtype=np.int32)
    q_k = np.asarray(inputs["q_kernel"], dtype=np.float32)
    kv_k = np.asarray(inputs["kv_kernel"], dtype=np.float32)
    o_k = np.asarray(inputs["o_kernel"], dtype=np.float32)
    gate_w = np.asarray(inputs["gate_w"], dtype=np.float32)
    up_w = np.asarray(inputs["up_w"], dtype=np.float32)
    down_w = np.asarray(inputs["down_w"], dtype=np.float32)

    xf = x.reshape(B * T, D)
    xbf = xf.astype(BF)
    premul = 1.0 + np.asarray(inputs["pre_attn_scale"], np.float32)
    postattn = (1.0 + np.asarray(inputs["post_attn_scale"],
                                 np.float32)).astype(BF)
    preffw = (1.0 + np.asarray(inputs["pre_ffw_scale"],
                               np.float32)).astype(BF)
    postffw = (1.0 + np.asarray(inputs["post_ffw_scale"],
                                np.float32)).astype(BF)
    qmul = ((1.0 + np.asarray(inputs["q_norm_scale"], np.float32))
            * np.float32(H ** -0.5))
    kmul = 1.0 + np.asarray(inputs["k_norm_scale"], np.float32)

    # rope tables with qk-norm col scales folded (slot 0,1 = q; 2 = k)
    frac = (2.0 * np.arange(H // 2, dtype=np.float32) / H)
    ts = ROPE_BASE ** frac
    sinu = seg.reshape(B * T)[:, None].astype(np.float32) / ts  # [BT, 64]
    sin, cos = np.sin(sinu), np.cos(sinu)
    muls = np.stack([qmul, qmul, kmul])                         # [3, 128]
    mf, ms = muls[:, 0:64], muls[:, 64:128]                     # [3, 64]
    tabA = (cos[:, None, :] * mf[None]).reshape(B * T, 192).astype(BF)
    tabB = (sin[:, None, :] * ms[None]).reshape(B * T, 192).astype(BF)
    tabC = (cos[:, None, :] * ms[None]).reshape(B * T, 192).astype(BF)
    tabD = (sin[:, None, :] * mf[None]).reshape(B * T, 192).astype(BF)

    # pre-attn rms row scale r_t (v-path only; q/k cancel in qk-norm)
    var = np.mean(xf * xf, axis=1) + EPS                        # [BT]
    rt = (1.0 / np.sqrt(var)).reshape(B, TB, 128).transpose(0, 2, 1)
    rt = np.ascontiguousarray(rt, dtype=np.float32)             # [B,128,TB]

    # canonical mask tiles -> per-(pair, qi) key-range rows
    ti = np.arange(128)[:, None]
    si = np.arange(128)[None, :]
    maskb = np.stack([
        np.full((128, 128), KMASK, np.float32),
        np.where(ti >= si, 0.0, KMASK).astype(np.float32),
        np.zeros((128, 128), np.float32),
        np.where(ti < si, 0.0, KMASK).astype(np.float32),
    ])
    # canonical tiles, transposed (keys on partitions), pre-divided by CAP
    maskq = np.stack([m.T / CAP for m in maskb], axis=1)
    maskq = np.ascontiguousarray(maskq.reshape(128, 4, 128)).astype(BF)

    # soft structural check of the actual attn_mask
    am = np.asarray(inputs["attn_mask"])
    tt = np.arange(T)
    sliding = (np.abs(tt[:, None] - tt[None, :]) <= WINDOW - 1)
    expected = am & sliding[None]
    ok = True
    for qb in range(min(2, TB)):
        for kb in range(qb + 1):
            blk = np.where(expected[0, qb * 128:(qb + 1) * 128,
                                    kb * 128:(kb + 1) * 128], 0.0, KMASK)
            if not np.array_equal(blk.astype(np.float32),
                                  maskb[_midx(qb, kb)]):
                ok = False
    if not ok:
        print("kernel.py WARNING: attn_mask does not match canonical "
              "causal+sliding structure; results may be wrong")

    in_maps = []
    for c in range(NCORES):
        qw_c = q_k[2 * c:2 * c + 2].transpose(1, 0, 2).reshape(D, 256)
        kw_c = kv_k[0, c]
        vw_c = kv_k[1, c]
        wqkv_c = np.concatenate([qw_c, kw_c, vw_c], axis=1)
        wqkv_c = (premul[:, None] * wqkv_c).astype(BF)
        ow_c = np.ascontiguousarray(o_k[2 * c:2 * c + 2].reshape(256, D)
                                    ).astype(BF)
        gw_c = np.ascontiguousarray(
            gate_w[:, 1024 * c:1024 * (c + 1)]).astype(BF)
        uw_c = np.ascontiguousarray(
            up_w[:, 1024 * c:1024 * (c + 1)]).astype(BF)
        dw_c = np.ascontiguousarray(
            down_w[1024 * c:1024 * (c + 1), :]).astype(BF)
        xsh_c = np.ascontiguousarray(np.concatenate(
            [xf[128 * c:128 * (c + 1)],
             xf[T + 128 * c:T + 128 * (c + 1)]], axis=0))
        in_maps.append({
            "xbf": xbf, "xsh": xsh_c, "wqkv": wqkv_c, "ow": ow_c,
            "gw": gw_c, "uw": uw_c, "dw": dw_c,
            "tabA": tabA, "tabB": tabB, "tabC": tabC, "tabD": tabD,
            "rt": rt, "maskq": maskq,
            "postattnmul": postattn, "preffwmul": preffw,
            "postffwmul": postffw,
        })
    return in_maps


def _assemble(results):
    out = np.empty((B, T, D), dtype=np.float32)
    for c in range(NCORES):
        r = results[c]["out"]
        out[0, 128 * c:128 * (c + 1)] = r[0:128]
        out[1, 128 * c:128 * (c + 1)] = r[128:256]
    return out


def kernel(**inputs) -> np.ndarray:
    from concourse import bass_utils
    nc = _get_nc()
    in_maps = _host_prep(inputs)
    r = bass_utils.run_bass_kernel_spmd(nc, in_maps,
                                        core_ids=list(range(NCORES)))
    return _assemble(r.results)
